# revision 1
# baseline (speedup 1.0000x reference)
"""Dense causal transformer attention block on 8 Trainium2 NeuronCores.

Problem: out = CausalAttention(RoPE(x@wq, x@wk), x@wv) @ wo
  x [2, 4096, 2048], 16 heads x 128 dim, fp32 I/O.

Sharding: tensor-parallel over heads. Core c owns heads {2c, 2c+1}:
  - computes qT/kT/vT ([head_dim, seq] layout) for its heads from the full
    (host-pre-transposed) xT, RoPE applied on-chip, V re-transposed to
    [seq, head_dim] on the PE (identity-matmul transpose),
  - runs causal attention in transposed form (scoresT = k @ qT so the
    softmax weights come out as the moving operand of the A@V matmul —
    no on-chip transpose of the probability matrix needed),
  - denominators via an all-ones [128,128] stationary matmul (comes out
    pre-broadcast across partitions),
  - computes its partial output projection o_local @ wo[rows of its heads].
Host sums the 8 partial outputs (the wo row-parallel all-reduce).

Compute dtype bf16 (PE 1 cycle/row), accumulation fp32 in PSUM.
"""
import sys

for _p in ("/opt/trn_rl_repo",):
    if _p not in sys.path:
        sys.path.insert(0, _p)

import numpy as np
import ml_dtypes
from contextlib import ExitStack

import concourse.bass as bass
import concourse.tile as tile
from concourse import bacc, mybir
from concourse import bass_utils

B, S, D = 2, 4096, 2048
H, DH = 16, 128
HALF = DH // 2
NC = 8
HPC = H // NC          # heads per core = 2
DOUT = HPC * DH        # 256 local proj width
ROPE_BASE = 10000.0
SCALE = 1.0 / float(np.sqrt(DH))
SQ = 512               # query tile (free dim of scoresT)
SKB = 128              # key block (partitions of scoresT)
KM = D // 128          # 16 contraction blocks
NSQ = S // SQ          # 8 query tiles per batch
BF = mybir.dt.bfloat16
F32 = mybir.dt.float32

_CACHED = {}


def _build():
    nc = bacc.Bacc("TRN2", target_bir_lowering=False, debug=False, num_devices=NC)

    xT = nc.dram_tensor("xT", [D, B * S], BF, kind="ExternalInput").ap()
    wq = nc.dram_tensor("wq", [D, DOUT], BF, kind="ExternalInput").ap()
    wk = nc.dram_tensor("wk", [D, DOUT], BF, kind="ExternalInput").ap()
    wv = nc.dram_tensor("wv", [D, DOUT], BF, kind="ExternalInput").ap()
    wo = nc.dram_tensor("wo", [DOUT, D], BF, kind="ExternalInput").ap()
    cosf = nc.dram_tensor("cosf", [DH, S], F32, kind="ExternalInput").ap()
    sins = nc.dram_tensor("sins", [DH, S], F32, kind="ExternalInput").ap()
    masks = nc.dram_tensor("masks", [SKB, 4 * SQ], BF, kind="ExternalInput").ap()
    ones = nc.dram_tensor("ones", [128, 128], BF, kind="ExternalInput").ap()
    ident = nc.dram_tensor("ident", [128, 128], BF, kind="ExternalInput").ap()
    outp = nc.dram_tensor("outp", [B * S, D], BF, kind="ExternalOutput").ap()

    with tile.TileContext(nc) as tc, ExitStack() as ctx:
        const = ctx.enter_context(tc.tile_pool(name="const", bufs=1))
        xpool = ctx.enter_context(tc.tile_pool(name="xpool", bufs=18))
        qkv = ctx.enter_context(tc.tile_pool(name="qkv", bufs=1))
        rope = ctx.enter_context(tc.tile_pool(name="rope", bufs=2))
        attn = ctx.enter_context(tc.tile_pool(name="attn", bufs=4))
        opool = ctx.enter_context(tc.tile_pool(name="opool", bufs=4))

        # ---- persistent constants -------------------------------------
        # Single packed tile per weight; DMA emitted inside the first tile
        # body (the Sync sequencer issues DMA instructions at ~0.6us each,
        # so emission order directly sets time-to-first-matmul).
        wq_sb = const.tile([128, KM * DOUT], BF, name="wq_sb")
        wk_sb = const.tile([128, KM * DOUT], BF, name="wk_sb")
        wv_sb = const.tile([128, KM * DOUT], BF, name="wv_sb")
        ones_sb = const.tile([128, 128], BF, name="ones_sb")
        nc.sync.dma_start(ones_sb[:], ones[:])
        id_sb = const.tile([128, 128], BF, name="id_sb")
        nc.sync.dma_start(id_sb[:], ident[:])
        # cos/sin/masks/wo are needed only after the first matmuls; their
        # DMAs are emitted inside the first tile's body so the t=0 x tiles
        # win the early DMA queue slots.
        cos_sb = const.tile([DH, S], F32, name="cos_sb")
        sin_sb = const.tile([DH, S], F32, name="sin_sb")  # rows 0-63 = -sin
        mask_sb = const.tile([SKB, 4 * SQ], BF, name="mask_sb")
        wo_sb = const.tile([128, HPC * D], BF, name="wo_sb")    # [p, h*2048+n]

        qT = [qkv.tile([128, S], BF, tag=f"qT{j}", name=f"qT{j}") for j in range(HPC)]
        kT = [qkv.tile([128, S], BF, tag=f"kT{j}", name=f"kT{j}") for j in range(HPC)]
        vsb = [qkv.tile([128, S], BF, tag=f"v{j}", name=f"v{j}") for j in range(HPC)]
        oT = [qkv.tile([128, S], BF, tag=f"oT{j}", name=f"oT{j}") for j in range(HPC)]

        with tc.tile_pool(name="psm", bufs=1, space="PSUM") as psm:
            # ---- fully merged per-t pipeline ---------------------------
            # One 8-bank PSUM pool shared by both batches:
            #   pqk (1 bank)   q then k accumulation, per head, sequential
            #   pv  (1 bank)   v accumulation + PE-transpose targets
            #   pscr(2x2 bank) attention score pairs [128,1024]
            #   po  (1 bank)   A@V accumulator
            #   pd  (1 bank)   denominator accumulator
            # Out-proj pf tiles share the pscr tag's slots.
            for b in range(B):
                for t in range(NSQ):
                    s0 = t * SQ
                    xbt = [xpool.tile([128, 8 * SQ], BF, tag="xb", bufs=3,
                                      name=f"xbt{hh}") for hh in range(2)]
                    for hh in range(2):
                        nc.sync.dma_start(
                            xbt[hh][:].rearrange("p (a n) -> p a n", n=SQ),
                            xT[hh * 1024:(hh + 1) * 1024,
                               b * S + s0: b * S + s0 + SQ]
                            .rearrange("(a p) n -> p a n", p=128))
                        if b == 0 and t == 0 and hh == 0:
                            nc.sync.dma_start(
                                wq_sb[:].rearrange("p (a n) -> p a n", n=DOUT),
                                wq.rearrange("(a p) n -> p a n", p=128))
                    if b == 0 and t == 0:
                        nc.sync.dma_start(
                            wk_sb[:].rearrange("p (a n) -> p a n", n=DOUT),
                            wk.rearrange("(a p) n -> p a n", p=128))
                        nc.sync.dma_start(
                            wv_sb[:].rearrange("p (a n) -> p a n", n=DOUT),
                            wv.rearrange("(a p) n -> p a n", p=128))
                        nc.sync.dma_start(cos_sb[:], cosf[:])
                        nc.sync.dma_start(sin_sb[:], sins[:])
                        nc.sync.dma_start(mask_sb[:], masks[:])
                        nc.sync.dma_start(
                            wo_sb[:].rearrange("p (a n) -> p a n", n=D),
                            wo.rearrange("(a p) n -> p a n", p=128))
                    # --- projections + RoPE, head by head ---------------
                    for j in range(HPC):
                        for w_sb, dstt in ((wq_sb, qT[j]), (wk_sb, kT[j])):
                            pp = psm.tile([128, SQ], F32, tag="pqk", name="pp")
                            for km in range(KM):
                                nc.tensor.matmul(
                                    pp[:],
                                    w_sb[:, km * DOUT + j * DH:
                                         km * DOUT + (j + 1) * DH],
                                    xbt[km // 8][:, (km % 8) * SQ:
                                                 (km % 8 + 1) * SQ],
                                    start=km == 0, stop=km == KM - 1)
                            rt = rope.tile([128, SQ], F32, tag="rot", name="rt")
                            nc.vector.tensor_mul(
                                rt[0:HALF, :], pp[HALF:128, :],
                                sin_sb[0:HALF, s0:s0 + SQ])
                            nc.vector.tensor_mul(
                                rt[HALF:128, :], pp[0:HALF, :],
                                sin_sb[HALF:128, s0:s0 + SQ])
                            m1 = rope.tile([128, SQ], F32, tag="m1", name="m1")
                            nc.vector.tensor_mul(m1[:], pp[:], cos_sb[:, s0:s0 + SQ])
                            nc.vector.tensor_add(dstt[:, s0:s0 + SQ], m1[:], rt[:])
                        pv = psm.tile([128, SQ], F32, tag="pv", name="pv")
                        for km in range(KM):
                            nc.tensor.matmul(
                                pv[:],
                                wv_sb[:, km * DOUT + j * DH:
                                      km * DOUT + (j + 1) * DH],
                                xbt[km // 8][:, (km % 8) * SQ:
                                             (km % 8 + 1) * SQ],
                                start=km == 0, stop=km == KM - 1)
                        vt = rope.tile([128, SQ], BF, tag="vt", name="vt")
                        nc.scalar.copy(vt[:], pv[:])
                        for sub in range(4):
                            ptr = psm.tile([128, 128], BF, tag="pv", name="ptr")
                            nc.tensor.transpose(
                                ptr[:], vt[:, sub * 128:(sub + 1) * 128], id_sb[:])
                            nc.vector.tensor_copy(
                                vsb[j][:, (4 * t + sub) * 128:(4 * t + sub + 1) * 128],
                                ptr[:])
                    # --- causal attention for this query tile -----------
                    for j in range(HPC):
                        nblk = 4 * t + 4
                        npair = nblk // 2
                        nquad = npair // 2
                        po = psm.tile([128, SQ], F32, tag="po", name="po")
                        pd = psm.tile([128, SQ], F32, tag="pd", name="pd")
                        prev_et = None
                        for p in range(npair):
                            pscr = psm.tile([128, 2 * SQ], F32, tag="pscr",
                                            bufs=2, name="pscr")
                            for h in range(2):
                                u = 2 * p + h
                                nc.tensor.matmul(
                                    pscr[:, h * SQ:(h + 1) * SQ],
                                    kT[j][:, u * SKB:(u + 1) * SKB],
                                    qT[j][:, s0:s0 + SQ], start=True, stop=True,
                                    skip_group_check=True)
                            et = attn.tile([128, 2 * SQ], BF, tag="et", bufs=4,
                                           name="et")
                            nc.scalar.activation(
                                et[:], pscr[:], mybir.ActivationFunctionType.Exp,
                                scale=SCALE)
                            if 2 * p >= 4 * t:  # pair on the diagonal band
                                r = 2 * p - 4 * t   # 0 or 2
                                nc.vector.tensor_mul(
                                    et[:], et[:],
                                    mask_sb[:, r * SQ:(r + 2) * SQ])
                            for h in range(2):
                                u = 2 * p + h
                                nc.tensor.matmul(
                                    po[:], vsb[j][:, u * 128:(u + 1) * 128],
                                    et[:, h * SQ:(h + 1) * SQ],
                                    start=u == 0, stop=u == nblk - 1)
                            if p % 2 == 1:
                                qi = p // 2
                                qs = attn.tile([128, 2 * SQ], BF, tag="qs",
                                               bufs=2, name="qs")
                                nc.vector.tensor_add(qs[:], prev_et[:], et[:])
                                qs2 = attn.tile([128, SQ], BF, tag="qs2",
                                                bufs=2, name="qs2")
                                nc.vector.tensor_add(
                                    qs2[:], qs[:, 0:SQ], qs[:, SQ:2 * SQ])
                                nc.tensor.matmul(
                                    pd[:], ones_sb[:], qs2[:],
                                    start=qi == 0, stop=qi == nquad - 1)
                            prev_et = et
                        rec = attn.tile([128, SQ], F32, tag="rec", bufs=2,
                                        name="rec")
                        nc.vector.reciprocal(rec[:], pd[:])
                        nc.vector.tensor_mul(oT[j][:, s0:s0 + SQ], po[:], rec[:])
                    # --- out-proj for the 4 seq blocks completed at t ----
                    # pf tiles borrow the pscr tag's 2-bank slots.
                    for m in range(4 * t, 4 * t + 4):
                        for n in range(D // 512):
                            pf = psm.tile([128, 512], F32, tag="pscr", bufs=2,
                                          name="pf")
                            for jj in range(HPC):
                                nc.tensor.matmul(
                                    pf[:], oT[jj][:, m * 128:(m + 1) * 128],
                                    wo_sb[:, jj * D + n * 512:
                                          jj * D + (n + 1) * 512],
                                    start=jj == 0, stop=jj == HPC - 1)
                            ob = opool.tile([128, 512], BF, tag="ob", name="ob")
                            if (m + n) % 2 == 0:
                                nc.vector.tensor_copy(ob[:], pf[:])
                            else:
                                nc.scalar.copy(ob[:], pf[:])
                            nc.sync.dma_start(
                                outp[b * S + m * 128: b * S + (m + 1) * 128,
                                     n * 512:(n + 1) * 512], ob[:])

    nc.compile()
    return nc


def _host_inputs(x, wq, wk, wv, wo, cos, sin):
    bf16 = ml_dtypes.bfloat16
    xT = np.ascontiguousarray(x.reshape(B * S, D).T).astype(bf16)

    cos = np.asarray(cos, dtype=np.float32)        # [S, 64]
    sin = np.asarray(sin, dtype=np.float32)
    cosf = np.ascontiguousarray(
        np.concatenate([cos, cos], axis=1).T)      # [128, S]
    sins = np.concatenate([-sin, sin], axis=1).T   # rows 0-63 negated
    sins = np.ascontiguousarray(sins)

    i = np.arange(SKB)[:, None]
    jj = np.arange(SQ)[None, :]
    masks = np.concatenate(
        [(i + r * SKB <= jj) for r in range(4)], axis=1).astype(bf16)
    ones = np.ones((128, 128), dtype=bf16)
    ident = np.eye(128, dtype=bf16)

    in_maps = []
    for c in range(NC):
        lo = c * DOUT
        in_maps.append({
            "xT": xT,
            "wq": np.ascontiguousarray(wq[:, lo:lo + DOUT]).astype(bf16),
            "wk": np.ascontiguousarray(wk[:, lo:lo + DOUT]).astype(bf16),
            "wv": np.ascontiguousarray(wv[:, lo:lo + DOUT]).astype(bf16),
            "wo": np.ascontiguousarray(wo[lo:lo + DOUT, :]).astype(bf16),
            "cosf": cosf,
            "sins": sins,
            "masks": masks,
            "ones": ones,
            "ident": ident,
        })
    return in_maps


def kernel(x, wq, wk, wv, wo, cos, sin, _trace=False, _tmpdir=None):
    if "nc" not in _CACHED:
        _CACHED["nc"] = _build()
    nc = _CACHED["nc"]
    in_maps = _host_inputs(
        np.asarray(x, dtype=np.float32), np.asarray(wq, dtype=np.float32),
        np.asarray(wk, dtype=np.float32), np.asarray(wv, dtype=np.float32),
        np.asarray(wo, dtype=np.float32), cos, sin)
    res = bass_utils.run_bass_kernel_spmd(
        nc, in_maps, core_ids=list(range(NC)), trace=_trace, tmpdir=_tmpdir)
    acc = np.zeros((B * S, D), dtype=np.float32)
    for c in range(NC):
        acc += res.results[c]["outp"].astype(np.float32)
    out = acc.reshape(B, S, D)
    if _trace:
        _CACHED["last_results"] = res
    return out



# revision 7
# speedup vs baseline: 1.0584x; 1.0584x over previous
"""Dense causal transformer attention block on 8 Trainium2 NeuronCores.

Problem: out = CausalAttention(RoPE(x@wq, x@wk), x@wv) @ wo
  x [2, 4096, 2048], 16 heads x 128 dim, fp32 I/O.

Sharding: tensor-parallel over heads. Core c owns heads {2c, 2c+1}:
  - computes qT/kT/vT ([head_dim, seq] layout) for its heads from the full
    (host-pre-transposed) xT, RoPE applied on-chip, V re-transposed to
    [seq, head_dim] on the PE (identity-matmul transpose),
  - runs causal attention in transposed form (scoresT = k @ qT so the
    softmax weights come out as the moving operand of the A@V matmul —
    no on-chip transpose of the probability matrix needed),
  - denominators via an all-ones [128,128] stationary matmul (comes out
    pre-broadcast across partitions),
  - computes its partial output projection o_local @ wo[rows of its heads].
Host sums the 8 partial outputs (the wo row-parallel all-reduce).

Compute dtype bf16 (PE 1 cycle/row), accumulation fp32 in PSUM.
"""
import sys

for _p in ("/opt/trn_rl_repo",):
    if _p not in sys.path:
        sys.path.insert(0, _p)

import numpy as np
import ml_dtypes
from contextlib import ExitStack

import concourse.bass as bass
import concourse.tile as tile
from concourse import bacc, mybir
from concourse import bass_utils

B, S, D = 2, 4096, 2048
H, DH = 16, 128
HALF = DH // 2
NC = 8
HPC = H // NC          # heads per core = 2
DOUT = HPC * DH        # 256 local proj width
ROPE_BASE = 10000.0
SCALE = 1.0 / float(np.sqrt(DH))
SQ = 512               # query tile (free dim of scoresT)
SKB = 128              # key block (partitions of scoresT)
KM = D // 128          # 16 contraction blocks
NSQ = S // SQ          # 8 query tiles per batch
BF = mybir.dt.bfloat16
F32 = mybir.dt.float32

_CACHED = {}


def _build():
    nc = bacc.Bacc("TRN2", target_bir_lowering=False, debug=False, num_devices=NC)

    xT = nc.dram_tensor("xT", [D, B * S], BF, kind="ExternalInput").ap()
    wq = nc.dram_tensor("wq", [D, DOUT], BF, kind="ExternalInput").ap()
    wk = nc.dram_tensor("wk", [D, DOUT], BF, kind="ExternalInput").ap()
    wv = nc.dram_tensor("wv", [D, DOUT], BF, kind="ExternalInput").ap()
    wo = nc.dram_tensor("wo", [DOUT, D], BF, kind="ExternalInput").ap()
    cosf = nc.dram_tensor("cosf", [DH, S], F32, kind="ExternalInput").ap()
    sins = nc.dram_tensor("sins", [DH, S], F32, kind="ExternalInput").ap()
    masks = nc.dram_tensor("masks", [SKB, 4 * SQ], BF, kind="ExternalInput").ap()
    ones = nc.dram_tensor("ones", [128, 128], BF, kind="ExternalInput").ap()
    ident = nc.dram_tensor("ident", [128, 128], BF, kind="ExternalInput").ap()
    outp = nc.dram_tensor("outp", [B * S, D], BF, kind="ExternalOutput").ap()

    with tile.TileContext(nc) as tc, ExitStack() as ctx:
        const = ctx.enter_context(tc.tile_pool(name="const", bufs=1))
        xpool = ctx.enter_context(tc.tile_pool(name="xpool", bufs=18))
        qkv = ctx.enter_context(tc.tile_pool(name="qkv", bufs=1))
        rope = ctx.enter_context(tc.tile_pool(name="rope", bufs=2))
        attn = ctx.enter_context(tc.tile_pool(name="attn", bufs=4))
        opool = ctx.enter_context(tc.tile_pool(name="opool", bufs=4))

        # ---- persistent constants -------------------------------------
        # Single packed tile per weight; DMA emitted inside the first tile
        # body (the Sync sequencer issues DMA instructions at ~0.6us each,
        # so emission order directly sets time-to-first-matmul).
        wq_sb = const.tile([128, KM * DOUT], BF, name="wq_sb")
        wk_sb = const.tile([128, KM * DOUT], BF, name="wk_sb")
        wv_sb = const.tile([128, KM * DOUT], BF, name="wv_sb")
        ones_sb = const.tile([128, 128], BF, name="ones_sb")
        nc.sync.dma_start(ones_sb[:], ones[:])
        id_sb = const.tile([128, 128], BF, name="id_sb")
        nc.sync.dma_start(id_sb[:], ident[:])
        # cos/sin/masks/wo are needed only after the first matmuls; their
        # DMAs are emitted inside the first tile's body so the t=0 x tiles
        # win the early DMA queue slots.
        cos_sb = const.tile([DH, S], F32, name="cos_sb")
        sin_sb = const.tile([DH, S], F32, name="sin_sb")  # rows 0-63 = -sin
        mask_sb = const.tile([SKB, 4 * SQ], BF, name="mask_sb")
        wo_sb = const.tile([128, HPC * D], BF, name="wo_sb")    # [p, h*2048+n]

        qT = [qkv.tile([128, S], BF, tag=f"qT{j}", name=f"qT{j}") for j in range(HPC)]
        kT = [qkv.tile([128, S], BF, tag=f"kT{j}", name=f"kT{j}") for j in range(HPC)]
        vsb = [qkv.tile([128, S], BF, tag=f"v{j}", name=f"v{j}") for j in range(HPC)]
        oT = [qkv.tile([128, S], BF, tag=f"oT{j}", name=f"oT{j}") for j in range(HPC)]

        def flat(i):
            return (i // NSQ, i % NSQ)

        xbts = {}

        def emit_x_dma(i):
            if i >= B * NSQ or i in xbts:
                return
            b_, t_ = flat(i)
            s0_ = t_ * SQ
            tiles = [xpool.tile([128, 8 * SQ], BF, tag=f"xb{hh}", bufs=2,
                                name=f"xbt{hh}") for hh in range(2)]
            for hh in range(2):
                nc.sync.dma_start(
                    tiles[hh][:].rearrange("p (a n) -> p a n", n=SQ),
                    xT[hh * 1024:(hh + 1) * 1024,
                       b_ * S + s0_: b_ * S + s0_ + SQ]
                    .rearrange("(a p) n -> p a n", p=128))
            xbts[i] = tiles

        with tc.tile_pool(name="psm", bufs=1, space="PSUM") as psm:
            # ---- fully merged per-t pipeline ---------------------------
            # One 8-bank PSUM pool shared by both batches:
            #   pqk (1 bank)   q then k accumulation, per head, sequential
            #   pv  (1 bank)   v accumulation + PE-transpose targets
            #   pscr(2x2 bank) attention score pairs [128,1024]
            #   po  (1 bank)   A@V accumulator
            #   pd  (1 bank)   denominator accumulator
            # Out-proj pf tiles share the pscr tag's slots.
            for b in range(B):
                for t in range(NSQ):
                    s0 = t * SQ
                    i_flat = b * NSQ + t
                    if i_flat == 0:
                        # startup: x(t0) + weights first, then x(t1)/x(t2)
                        # prefetch, then the late-needed constants.
                        emit_x_dma(0)
                        nc.sync.dma_start(
                            wq_sb[:].rearrange("p (a n) -> p a n", n=DOUT),
                            wq.rearrange("(a p) n -> p a n", p=128))
                        nc.sync.dma_start(
                            wk_sb[:].rearrange("p (a n) -> p a n", n=DOUT),
                            wk.rearrange("(a p) n -> p a n", p=128))
                        nc.sync.dma_start(
                            wv_sb[:].rearrange("p (a n) -> p a n", n=DOUT),
                            wv.rearrange("(a p) n -> p a n", p=128))
                        emit_x_dma(1)
                        nc.sync.dma_start(cos_sb[:], cosf[:])
                        nc.sync.dma_start(sin_sb[:], sins[:])
                        nc.sync.dma_start(mask_sb[:], masks[:])
                        nc.sync.dma_start(
                            wo_sb[:].rearrange("p (a n) -> p a n", n=D),
                            wo.rearrange("(a p) n -> p a n", p=128))
                    else:
                        emit_x_dma(i_flat + 1)
                    xbt = xbts.pop(i_flat)
                    # --- projections + RoPE, head by head ---------------
                    for j in range(HPC):
                        for w_sb, dstt in ((wq_sb, qT[j]), (wk_sb, kT[j])):
                            pp = psm.tile([128, SQ], F32, tag="pqk", name="pp")
                            for km in range(KM):
                                nc.tensor.matmul(
                                    pp[:],
                                    w_sb[:, km * DOUT + j * DH:
                                         km * DOUT + (j + 1) * DH],
                                    xbt[km // 8][:, (km % 8) * SQ:
                                                 (km % 8 + 1) * SQ],
                                    start=km == 0, stop=km == KM - 1)
                            rt = rope.tile([128, SQ], F32, tag="rot", name="rt")
                            nc.vector.tensor_mul(
                                rt[0:HALF, :], pp[HALF:128, :],
                                sin_sb[0:HALF, s0:s0 + SQ])
                            nc.vector.tensor_mul(
                                rt[HALF:128, :], pp[0:HALF, :],
                                sin_sb[HALF:128, s0:s0 + SQ])
                            m1 = rope.tile([128, SQ], F32, tag="m1", name="m1")
                            nc.vector.tensor_mul(m1[:], pp[:], cos_sb[:, s0:s0 + SQ])
                            nc.vector.tensor_add(dstt[:, s0:s0 + SQ], m1[:], rt[:])
                        pv = psm.tile([128, SQ], F32, tag="pv", name="pv")
                        for km in range(KM):
                            nc.tensor.matmul(
                                pv[:],
                                wv_sb[:, km * DOUT + j * DH:
                                      km * DOUT + (j + 1) * DH],
                                xbt[km // 8][:, (km % 8) * SQ:
                                             (km % 8 + 1) * SQ],
                                start=km == 0, stop=km == KM - 1)
                        vt = rope.tile([128, SQ], BF, tag="vt", name="vt")
                        nc.scalar.copy(vt[:], pv[:])
                        for sub in range(4):
                            ptr = psm.tile([128, 128], BF, tag="pv", name="ptr")
                            nc.tensor.transpose(
                                ptr[:], vt[:, sub * 128:(sub + 1) * 128], id_sb[:])
                            nc.vector.tensor_copy(
                                vsb[j][:, (4 * t + sub) * 128:(4 * t + sub + 1) * 128],
                                ptr[:])
                    # --- causal attention for this query tile -----------
                    for j in range(HPC):
                        nblk = 4 * t + 4
                        npair = nblk // 2
                        nquad = npair // 2
                        po = psm.tile([128, SQ], F32, tag="po", name="po")
                        pd = psm.tile([128, SQ], F32, tag="pd", name="pd")
                        prev_et = None
                        for p in range(npair):
                            pscr = psm.tile([128, 2 * SQ], F32, tag="pscr",
                                            bufs=2, name="pscr")
                            for h in range(2):
                                u = 2 * p + h
                                nc.tensor.matmul(
                                    pscr[:, h * SQ:(h + 1) * SQ],
                                    kT[j][:, u * SKB:(u + 1) * SKB],
                                    qT[j][:, s0:s0 + SQ], start=True, stop=True,
                                    skip_group_check=True)
                            et = attn.tile([128, 2 * SQ], BF, tag="et", bufs=4,
                                           name="et")
                            nc.scalar.activation(
                                et[:], pscr[:], mybir.ActivationFunctionType.Exp,
                                scale=SCALE)
                            if 2 * p >= 4 * t:  # pair on the diagonal band
                                r = 2 * p - 4 * t   # 0 or 2
                                nc.gpsimd.tensor_mul(
                                    et[:], et[:],
                                    mask_sb[:, r * SQ:(r + 2) * SQ])
                            for h in range(2):
                                u = 2 * p + h
                                nc.tensor.matmul(
                                    po[:], vsb[j][:, u * 128:(u + 1) * 128],
                                    et[:, h * SQ:(h + 1) * SQ],
                                    start=u == 0, stop=u == nblk - 1)
                            if p % 2 == 1:
                                qi = p // 2
                                qs = attn.tile([128, 2 * SQ], BF, tag="qs",
                                               bufs=2, name="qs")
                                nc.vector.tensor_add(qs[:], prev_et[:], et[:])
                                qs2 = attn.tile([128, SQ], BF, tag="qs2",
                                                bufs=2, name="qs2")
                                nc.vector.tensor_add(
                                    qs2[:], qs[:, 0:SQ], qs[:, SQ:2 * SQ])
                                nc.tensor.matmul(
                                    pd[:], ones_sb[:], qs2[:],
                                    start=qi == 0, stop=qi == nquad - 1)
                            prev_et = et
                        rec = attn.tile([128, SQ], F32, tag="rec", bufs=2,
                                        name="rec")
                        nc.vector.reciprocal_approx_fast(rec[:], pd[:])
                        nc.vector.tensor_mul(oT[j][:, s0:s0 + SQ], po[:], rec[:])
                    # --- out-proj for the 4 seq blocks completed at t ----
                    # pf tiles borrow the pscr tag's 2-bank slots; the 4
                    # n-blocks are gathered in one [128, 2048] SBUF tile so
                    # each m-block is a single output DMA.
                    for m in range(4 * t, 4 * t + 4):
                        ob = opool.tile([128, D], BF, tag="ob", bufs=2,
                                        name="ob")
                        for n in range(D // 512):
                            pf = psm.tile([128, 512], F32, tag="pscr", bufs=2,
                                          name="pf")
                            for jj in range(HPC):
                                nc.tensor.matmul(
                                    pf[:], oT[jj][:, m * 128:(m + 1) * 128],
                                    wo_sb[:, jj * D + n * 512:
                                          jj * D + (n + 1) * 512],
                                    start=jj == 0, stop=jj == HPC - 1)
                            if (m + n) % 2 == 0:
                                nc.vector.tensor_copy(
                                    ob[:, n * 512:(n + 1) * 512], pf[:])
                            else:
                                nc.scalar.copy(
                                    ob[:, n * 512:(n + 1) * 512], pf[:])
                        nc.sync.dma_start(
                            outp[b * S + m * 128: b * S + (m + 1) * 128, :],
                            ob[:])

    nc.compile()
    return nc


def _host_inputs(x, wq, wk, wv, wo, cos, sin):
    bf16 = ml_dtypes.bfloat16
    xT = np.ascontiguousarray(x.reshape(B * S, D).T).astype(bf16)

    cos = np.asarray(cos, dtype=np.float32)        # [S, 64]
    sin = np.asarray(sin, dtype=np.float32)
    cosf = np.ascontiguousarray(
        np.concatenate([cos, cos], axis=1).T)      # [128, S]
    sins = np.concatenate([-sin, sin], axis=1).T   # rows 0-63 negated
    sins = np.ascontiguousarray(sins)

    i = np.arange(SKB)[:, None]
    jj = np.arange(SQ)[None, :]
    masks = np.concatenate(
        [(i + r * SKB <= jj) for r in range(4)], axis=1).astype(bf16)
    ones = np.ones((128, 128), dtype=bf16)
    ident = np.eye(128, dtype=bf16)

    in_maps = []
    for c in range(NC):
        lo = c * DOUT
        in_maps.append({
            "xT": xT,
            "wq": np.ascontiguousarray(wq[:, lo:lo + DOUT]).astype(bf16),
            "wk": np.ascontiguousarray(wk[:, lo:lo + DOUT]).astype(bf16),
            "wv": np.ascontiguousarray(wv[:, lo:lo + DOUT]).astype(bf16),
            "wo": np.ascontiguousarray(wo[lo:lo + DOUT, :]).astype(bf16),
            "cosf": cosf,
            "sins": sins,
            "masks": masks,
            "ones": ones,
            "ident": ident,
        })
    return in_maps


def kernel(x, wq, wk, wv, wo, cos, sin, _trace=False, _tmpdir=None):
    if "nc" not in _CACHED:
        _CACHED["nc"] = _build()
    nc = _CACHED["nc"]
    in_maps = _host_inputs(
        np.asarray(x, dtype=np.float32), np.asarray(wq, dtype=np.float32),
        np.asarray(wk, dtype=np.float32), np.asarray(wv, dtype=np.float32),
        np.asarray(wo, dtype=np.float32), cos, sin)
    res = bass_utils.run_bass_kernel_spmd(
        nc, in_maps, core_ids=list(range(NC)), trace=_trace, tmpdir=_tmpdir)
    acc = np.zeros((B * S, D), dtype=np.float32)
    for c in range(NC):
        acc += res.results[c]["outp"].astype(np.float32)
    out = acc.reshape(B, S, D)
    if _trace:
        _CACHED["last_results"] = res
    return out



# revision 16
# speedup vs baseline: 1.1353x; 1.0727x over previous
"""Dense causal transformer attention block on 8 Trainium2 NeuronCores.

Problem: out = CausalAttention(RoPE(x@wq, x@wk), x@wv) @ wo
  x [2, 4096, 2048], 16 heads x 128 dim, fp32 I/O.

Sharding: tensor-parallel over heads. Core c owns heads {2c, 2c+1}:
  - computes qT/kT/vT ([head_dim, seq] layout) for its heads from the full
    (host-pre-transposed) xT, RoPE applied on-chip, V re-transposed to
    [seq, head_dim] on the PE (identity-matmul transpose),
  - runs causal attention in transposed form (scoresT = k @ qT so the
    softmax weights come out as the moving operand of the A@V matmul —
    no on-chip transpose of the probability matrix needed),
  - denominators via an all-ones [128,128] stationary matmul (comes out
    pre-broadcast across partitions),
  - computes its partial output projection o_local @ wo[rows of its heads].
Host sums the 8 partial outputs (the wo row-parallel all-reduce).

Compute dtype bf16 (PE 1 cycle/row), accumulation fp32 in PSUM.
"""
import sys

for _p in ("/opt/trn_rl_repo",):
    if _p not in sys.path:
        sys.path.insert(0, _p)

import numpy as np
import ml_dtypes
from contextlib import ExitStack

import concourse.bass as bass
import concourse.tile as tile
from concourse import bacc, mybir
from concourse import bass_utils

B, S, D = 2, 4096, 2048
H, DH = 16, 128
HALF = DH // 2
NC = 8
HPC = H // NC          # heads per core = 2
DOUT = HPC * DH        # 256 local proj width
ROPE_BASE = 10000.0
SCALE = 1.0 / float(np.sqrt(DH))
SQ = 512               # query tile (free dim of scoresT)
SKB = 128              # key block (partitions of scoresT)
KM = D // 128          # 16 contraction blocks
NSQ = S // SQ          # 8 query tiles per batch
BF = mybir.dt.bfloat16
F32 = mybir.dt.float32

_CACHED = {}


def _build():
    nc = bacc.Bacc("TRN2", target_bir_lowering=False, debug=False, num_devices=NC)

    xT = nc.dram_tensor("xT", [D, B * S], BF, kind="ExternalInput").ap()
    wq = nc.dram_tensor("wq", [D, DOUT], BF, kind="ExternalInput").ap()
    wk = nc.dram_tensor("wk", [D, DOUT], BF, kind="ExternalInput").ap()
    wv = nc.dram_tensor("wv", [D, DOUT], BF, kind="ExternalInput").ap()
    wo = nc.dram_tensor("wo", [DOUT, D], BF, kind="ExternalInput").ap()
    cosf = nc.dram_tensor("cosf", [DH, S], BF, kind="ExternalInput").ap()
    sins = nc.dram_tensor("sins", [DH, S], BF, kind="ExternalInput").ap()
    masks = nc.dram_tensor("masks", [SKB, 4 * SQ], BF, kind="ExternalInput").ap()
    ones = nc.dram_tensor("ones", [128, 128], BF, kind="ExternalInput").ap()
    ident = nc.dram_tensor("ident", [128, 128], BF, kind="ExternalInput").ap()
    outp = nc.dram_tensor("outp", [B * S, D], BF, kind="ExternalOutput").ap()

    with tile.TileContext(nc) as tc, ExitStack() as ctx:
        const = ctx.enter_context(tc.tile_pool(name="const", bufs=1))
        xpool = ctx.enter_context(tc.tile_pool(name="xpool", bufs=18))
        qkv = ctx.enter_context(tc.tile_pool(name="qkv", bufs=1))
        rope = ctx.enter_context(tc.tile_pool(name="rope", bufs=2))
        attn = ctx.enter_context(tc.tile_pool(name="attn", bufs=4))
        opool = ctx.enter_context(tc.tile_pool(name="opool", bufs=4))

        # ---- persistent constants -------------------------------------
        # Single packed tile per weight; DMA emitted inside the first tile
        # body (the Sync sequencer issues DMA instructions at ~0.6us each,
        # so emission order directly sets time-to-first-matmul).
        wq_sb = const.tile([128, KM * DOUT], BF, name="wq_sb")
        wk_sb = const.tile([128, KM * DOUT], BF, name="wk_sb")
        wv_sb = const.tile([128, KM * DOUT], BF, name="wv_sb")
        ones_sb = const.tile([128, 128], BF, name="ones_sb")
        nc.sync.dma_start(ones_sb[:], ones[:])
        id_sb = const.tile([128, 128], BF, name="id_sb")
        nc.sync.dma_start(id_sb[:], ident[:])
        # cos/sin/masks/wo are needed only after the first matmuls; their
        # DMAs are emitted inside the first tile's body so the t=0 x tiles
        # win the early DMA queue slots.
        cos_sb = const.tile([DH, S], BF, name="cos_sb")
        sin_sb = const.tile([DH, S], BF, name="sin_sb")  # rows 0-63 = -sin
        mask_sb = const.tile([SKB, 4 * SQ], BF, name="mask_sb")
        wo_sb = const.tile([128, HPC * D], BF, name="wo_sb")    # [p, h*2048+n]

        qT = [qkv.tile([128, S], BF, tag=f"qT{j}", name=f"qT{j}") for j in range(HPC)]
        kT = [qkv.tile([128, S], BF, tag=f"kT{j}", name=f"kT{j}") for j in range(HPC)]
        vsb = [qkv.tile([128, S], BF, tag=f"v{j}", name=f"v{j}") for j in range(HPC)]
        oT = [qkv.tile([128, S], BF, tag=f"oT{j}", name=f"oT{j}") for j in range(HPC)]

        def flat(i):
            return (i // NSQ, i % NSQ)

        xbts = {}

        def emit_x_dma(i, eng=None, halves=False):
            if i >= B * NSQ or i in xbts:
                return
            b_, t_ = flat(i)
            s0_ = t_ * SQ
            tiles = [xpool.tile([128, 8 * SQ], BF, tag=f"xb{hh}", bufs=2,
                                name=f"xbt{hh}") for hh in range(2)]
            xbts[i] = tiles
            e = eng or nc.sync
            if not halves:
                for hh in range(2):
                    e.dma_start(
                        tiles[hh][:].rearrange("p (a n) -> p a n", n=SQ),
                        xT[hh * 1024:(hh + 1) * 1024,
                           b_ * S + s0_: b_ * S + s0_ + SQ]
                        .rearrange("(a p) n -> p a n", p=128))
                return

            # split into 4 half-tile DMA closures so the caller can
            # interleave them with the weight DMAs (first proj matmuls
            # start after only 512 rows of x have landed)
            def part(hh, ha):
                e.dma_start(
                    tiles[hh][:, ha * 4 * SQ:(ha + 1) * 4 * SQ]
                    .rearrange("p (a n) -> p a n", n=SQ),
                    xT[hh * 1024 + ha * 512: hh * 1024 + (ha + 1) * 512,
                       b_ * S + s0_: b_ * S + s0_ + SQ]
                    .rearrange("(a p) n -> p a n", p=128))
            return part

        with tc.tile_pool(name="psm", bufs=1, space="PSUM") as psm:
            # ---- fully merged per-t pipeline ---------------------------
            # One 8-bank PSUM pool shared by both batches:
            #   pqk (1 bank)   q then k accumulation, per head, sequential
            #   pv  (1 bank)   v accumulation + PE-transpose targets
            #   pscr(2x2 bank) attention score pairs [128,1024]
            #   po  (1 bank)   A@V accumulator
            #   pd  (1 bank)   denominator accumulator
            # Out-proj pf tiles share the pscr tag's slots.
            for b in range(B):
                for t in range(NSQ):
                    s0 = t * SQ
                    i_flat = b * NSQ + t
                    if i_flat == 0:
                        # startup: interleave x(t0) halves with wq halves so
                        # the first proj chain starts after ~0.5 MB of DMA;
                        # later-needed constants follow in need order.
                        xpart = emit_x_dma(0, halves=True)

                        def wq_half(wh):
                            nc.sync.dma_start(
                                wq_sb[:, wh * 8 * DOUT:(wh + 1) * 8 * DOUT]
                                .rearrange("p (a n) -> p a n", n=DOUT),
                                wq[wh * 1024:(wh + 1) * 1024, :]
                                .rearrange("(a p) n -> p a n", p=128))
                        xpart(0, 0)
                        wq_half(0)
                        xpart(0, 1)
                        wq_half(1)
                        xpart(1, 0)
                        xpart(1, 1)
                        nc.sync.dma_start(cos_sb[:], cosf[:])
                        nc.sync.dma_start(sin_sb[:], sins[:])
                        nc.sync.dma_start(
                            wk_sb[:].rearrange("p (a n) -> p a n", n=DOUT),
                            wk.rearrange("(a p) n -> p a n", p=128))
                        nc.sync.dma_start(
                            wv_sb[:].rearrange("p (a n) -> p a n", n=DOUT),
                            wv.rearrange("(a p) n -> p a n", p=128))
                        nc.sync.dma_start(mask_sb[:], masks[:])
                        emit_x_dma(1)
                        nc.sync.dma_start(
                            wo_sb[:].rearrange("p (a n) -> p a n", n=D),
                            wo.rearrange("(a p) n -> p a n", p=128))
                    else:
                        emit_x_dma(i_flat + 1, eng=nc.scalar)
                    xbt = xbts.pop(i_flat)
                    # --- projections + RoPE, head by head ---------------
                    for j in range(HPC):
                        for w_sb, dstt in ((wq_sb, qT[j]), (wk_sb, kT[j])):
                            pp = psm.tile([128, SQ], F32, tag="pqk", name="pp")
                            for km in range(KM):
                                nc.tensor.matmul(
                                    pp[:],
                                    w_sb[:, km * DOUT + j * DH:
                                         km * DOUT + (j + 1) * DH],
                                    xbt[km // 8][:, (km % 8) * SQ:
                                                 (km % 8 + 1) * SQ],
                                    start=km == 0, stop=km == KM - 1)
                            rt = rope.tile([128, SQ], F32, tag="rot", name="rt")
                            nc.vector.tensor_mul(
                                rt[0:HALF, :], pp[HALF:128, :],
                                sin_sb[0:HALF, s0:s0 + SQ])
                            nc.vector.tensor_mul(
                                rt[HALF:128, :], pp[0:HALF, :],
                                sin_sb[HALF:128, s0:s0 + SQ])
                            m1 = rope.tile([128, SQ], F32, tag="m1", name="m1")
                            nc.vector.tensor_mul(m1[:], pp[:], cos_sb[:, s0:s0 + SQ])
                            nc.vector.tensor_add(dstt[:, s0:s0 + SQ], m1[:], rt[:])
                        pv = psm.tile([128, SQ], F32, tag="pv", name="pv")
                        for km in range(KM):
                            nc.tensor.matmul(
                                pv[:],
                                wv_sb[:, km * DOUT + j * DH:
                                      km * DOUT + (j + 1) * DH],
                                xbt[km // 8][:, (km % 8) * SQ:
                                             (km % 8 + 1) * SQ],
                                start=km == 0, stop=km == KM - 1)
                        vt = rope.tile([128, SQ], BF, tag="vt", name="vt")
                        nc.scalar.copy(vt[:], pv[:])
                        for sub in range(4):
                            ptr = psm.tile([128, 128], BF, tag="pv", name="ptr")
                            nc.tensor.transpose(
                                ptr[:], vt[:, sub * 128:(sub + 1) * 128], id_sb[:])
                            nc.vector.tensor_copy(
                                vsb[j][:, (4 * t + sub) * 128:(4 * t + sub + 1) * 128],
                                ptr[:])
                    # --- causal attention for this query tile -----------
                    for j in range(HPC):
                        nblk = 4 * t + 4
                        npair = nblk // 2
                        nquad = npair // 2
                        po = psm.tile([128, SQ], F32, tag="po", name="po")
                        pd = psm.tile([128, SQ], F32, tag="pd", name="pd")
                        # Pair order: one mask-free pair first (its et needs
                        # only the exp), then the diagonal (masked) pairs so
                        # their exp+mask latency hides behind the mask-free
                        # tail, then the remaining old-KV pairs.
                        if t == 0:
                            order = [0, 1]
                        else:
                            order = [0, 2 * t, 2 * t + 1] + list(range(1, 2 * t))
                        prev_et = None
                        for idx, p in enumerate(order):
                            pscr = psm.tile([128, 2 * SQ], F32, tag="pscr",
                                            bufs=2, name="pscr")
                            for h in range(2):
                                u = 2 * p + h
                                nc.tensor.matmul(
                                    pscr[:, h * SQ:(h + 1) * SQ],
                                    kT[j][:, u * SKB:(u + 1) * SKB],
                                    qT[j][:, s0:s0 + SQ], start=True, stop=True,
                                    skip_group_check=True)
                            et = attn.tile([128, 2 * SQ], BF, tag="et", bufs=4,
                                           name="et")
                            nc.scalar.activation(
                                et[:], pscr[:], mybir.ActivationFunctionType.Exp,
                                scale=SCALE)
                            if 2 * p >= 4 * t:  # pair on the diagonal band
                                r = 2 * p - 4 * t   # 0 or 2
                                nc.gpsimd.tensor_mul(
                                    et[:, 0:SQ], et[:, 0:SQ],
                                    mask_sb[:, r * SQ:(r + 1) * SQ])
                                nc.vector.tensor_mul(
                                    et[:, SQ:2 * SQ], et[:, SQ:2 * SQ],
                                    mask_sb[:, (r + 1) * SQ:(r + 2) * SQ])
                            for h in range(2):
                                u = 2 * p + h
                                nc.tensor.matmul(
                                    po[:], vsb[j][:, u * 128:(u + 1) * 128],
                                    et[:, h * SQ:(h + 1) * SQ],
                                    start=idx == 0 and h == 0,
                                    stop=idx == npair - 1 and h == 1)
                            if idx % 2 == 1:
                                qi = idx // 2
                                qs = attn.tile([128, 2 * SQ], BF, tag="qs",
                                               bufs=2, name="qs")
                                nc.vector.tensor_add(qs[:], prev_et[:], et[:])
                                qs2 = attn.tile([128, SQ], BF, tag="qs2",
                                                bufs=2, name="qs2")
                                nc.vector.tensor_add(
                                    qs2[:], qs[:, 0:SQ], qs[:, SQ:2 * SQ])
                                nc.tensor.matmul(
                                    pd[:], ones_sb[:], qs2[:],
                                    start=qi == 0, stop=qi == nquad - 1)
                            prev_et = et
                        rec = attn.tile([128, SQ], F32, tag="rec", bufs=2,
                                        name="rec")
                        nc.vector.reciprocal_approx_fast(rec[:], pd[:])
                        nc.vector.tensor_mul(oT[j][:, s0:s0 + SQ], po[:], rec[:])
                    # --- out-proj for the 4 seq blocks completed at t ----
                    # pf tiles borrow the pscr tag's 2-bank slots; the 4
                    # n-blocks are gathered in one [128, 2048] SBUF tile so
                    # each m-block is a single output DMA.
                    last_body = i_flat == B * NSQ - 1
                    for m in range(4 * t, 4 * t + 4):
                        ob = opool.tile([128, D], BF, tag="ob", bufs=2,
                                        name="ob")
                        for n in range(D // 512):
                            pf = psm.tile([128, 512], F32, tag="pscr", bufs=2,
                                          name="pf")
                            for jj in range(HPC):
                                nc.tensor.matmul(
                                    pf[:], oT[jj][:, m * 128:(m + 1) * 128],
                                    wo_sb[:, jj * D + n * 512:
                                          jj * D + (n + 1) * 512],
                                    start=jj == 0, stop=jj == HPC - 1)
                            if (m + n) % 2 == 0:
                                nc.vector.tensor_copy(
                                    ob[:, n * 512:(n + 1) * 512], pf[:])
                            else:
                                nc.scalar.copy(
                                    ob[:, n * 512:(n + 1) * 512], pf[:])
                            if last_body:
                                # tail: per-block DMAs overlap the remaining
                                # copies instead of waiting for all four
                                nc.sync.dma_start(
                                    outp[b * S + m * 128:
                                         b * S + (m + 1) * 128,
                                         n * 512:(n + 1) * 512],
                                    ob[:, n * 512:(n + 1) * 512])
                        if not last_body:
                            nc.sync.dma_start(
                                outp[b * S + m * 128: b * S + (m + 1) * 128, :],
                                ob[:])

    nc.compile()
    return nc


def _host_inputs(x, wq, wk, wv, wo, cos, sin):
    bf16 = ml_dtypes.bfloat16
    xT = np.ascontiguousarray(x.reshape(B * S, D).T).astype(bf16)

    cos = np.asarray(cos, dtype=np.float32)        # [S, 64]
    sin = np.asarray(sin, dtype=np.float32)
    cosf = np.ascontiguousarray(
        np.concatenate([cos, cos], axis=1).T).astype(bf16)   # [128, S]
    sins = np.concatenate([-sin, sin], axis=1).T   # rows 0-63 negated
    sins = np.ascontiguousarray(sins).astype(bf16)

    i = np.arange(SKB)[:, None]
    jj = np.arange(SQ)[None, :]
    masks = np.concatenate(
        [(i + r * SKB <= jj) for r in range(4)], axis=1).astype(bf16)
    ones = np.ones((128, 128), dtype=bf16)
    ident = np.eye(128, dtype=bf16)

    in_maps = []
    for c in range(NC):
        lo = c * DOUT
        in_maps.append({
            "xT": xT,
            "wq": np.ascontiguousarray(wq[:, lo:lo + DOUT]).astype(bf16),
            "wk": np.ascontiguousarray(wk[:, lo:lo + DOUT]).astype(bf16),
            "wv": np.ascontiguousarray(wv[:, lo:lo + DOUT]).astype(bf16),
            "wo": np.ascontiguousarray(wo[lo:lo + DOUT, :]).astype(bf16),
            "cosf": cosf,
            "sins": sins,
            "masks": masks,
            "ones": ones,
            "ident": ident,
        })
    return in_maps


def kernel(x, wq, wk, wv, wo, cos, sin, _trace=False, _tmpdir=None):
    if "nc" not in _CACHED:
        _CACHED["nc"] = _build()
    nc = _CACHED["nc"]
    in_maps = _host_inputs(
        np.asarray(x, dtype=np.float32), np.asarray(wq, dtype=np.float32),
        np.asarray(wk, dtype=np.float32), np.asarray(wv, dtype=np.float32),
        np.asarray(wo, dtype=np.float32), cos, sin)
    res = bass_utils.run_bass_kernel_spmd(
        nc, in_maps, core_ids=list(range(NC)), trace=_trace, tmpdir=_tmpdir)
    acc = np.zeros((B * S, D), dtype=np.float32)
    for c in range(NC):
        acc += res.results[c]["outp"].astype(np.float32)
    out = acc.reshape(B, S, D)
    if _trace:
        _CACHED["last_results"] = res
    return out

